# revision 1
# baseline (speedup 1.0000x reference)
"""Cross-attention kernel for 8 Trainium2 NeuronCores.

Sharding: data-parallel over batch (B=2) x tensor-parallel over heads
(16 heads -> 4 groups of 4 heads).  Core c handles batch c//4, head
group c%4.  Each core computes, for its 4 heads:
    Q^T = Wq_g^T x_b^T        [256, 2048]   (d-on-partitions layout)
    K^T = Wk_g^T y_b^T        [256, 2048]
    V   = y_b Wv_g            [2048, 256]   (n-on-partitions layout)
    S^T_h = K_h Q_h^T / 8; P^T = exp(S^T)
    O^T_h (+row sums via a ones-column in V) = [V_h|1]^T P^T
    partial = (O^T/rowsum)^T Wp_g           [2048, 1024]
The 4 partials per batch are summed on the host and bp is added.

Matmuls run as float32r (full-rate fp32 on the PE at moving-dim>=256).
Head pairs share the PE array via tile_position row packing for the
S^T matmuls; two query blocks are interleaved per head pair so the
exp (ScalarE) chain of one block hides the semaphore latency of the
other.
"""

import numpy as np

B = 2
N = 2048          # query sequence length
M = 2048          # key sequence length
DIM = 1024
HEAD_DIM = 64
SCALE = HEAD_DIM ** -0.5
NCORES = 8
GH = 4            # heads per core
J = GH * HEAD_DIM # 256 projected columns per core
KC = DIM // 128   # 8 contraction chunks
NT = M // 128     # 16 key tiles
IBS = 512         # i-block size
IB = N // IBS     # 4 i-blocks

PACK_S = True

_NC = None


def _build():
    from contextlib import ExitStack

    import concourse.bass as bass
    import concourse.tile as tile
    from concourse import bacc, mybir
    from concourse.bass import ts, ds
    from concourse.masks import make_identity

    f32 = mybir.dt.float32
    f32r = mybir.dt.float32r
    Exp = mybir.ActivationFunctionType.Exp

    nc = bacc.Bacc("TRN2", target_bir_lowering=False, debug=False,
                   num_devices=NCORES)
    xT = nc.dram_tensor("xT", [DIM, N], f32r, kind="ExternalInput").ap()
    yT = nc.dram_tensor("yT", [DIM, M], f32r, kind="ExternalInput").ap()
    wq = nc.dram_tensor("wq", [DIM, J], f32r, kind="ExternalInput").ap()
    wk = nc.dram_tensor("wk", [DIM, J], f32r, kind="ExternalInput").ap()
    wv = nc.dram_tensor("wv", [DIM, J], f32r, kind="ExternalInput").ap()
    wp = nc.dram_tensor("wp", [J, DIM], f32r, kind="ExternalInput").ap()
    out = nc.dram_tensor("out", [N, DIM], f32, kind="ExternalOutput").ap()

    with tile.TileContext(nc) as tc, ExitStack() as top:
        wpool = top.enter_context(tc.tile_pool(name="weights", bufs=1))
        wq_sb = wpool.tile([128, KC, J], f32r, name="wq_sb")
        wk_sb = wpool.tile([128, KC, J], f32r, name="wk_sb")
        wv_sb = wpool.tile([128, KC, J], f32r, name="wv_sb")
        wp_sb = wpool.tile([128, 2, DIM], f32r, name="wp_sb")
        # wq + x stream on the SP (sync) HWDGE queue; everything else on
        # the Activation HWDGE queue so Q's inputs aren't stuck behind 12MB
        wq_r = wq.rearrange("(c p) j -> p c j", p=128)
        nc.sync.dma_start(wq_sb[:, 0, :], wq_r[:, 0, :])
        nc.scalar.dma_start(wk_sb, wk.rearrange("(c p) j -> p c j", p=128))
        nc.scalar.dma_start(wv_sb, wv.rearrange("(c p) j -> p c j", p=128))
        nc.scalar.dma_start(wp_sb, wp.rearrange("(t p) c -> p t c", p=128))

        big = top.enter_context(tc.tile_pool(name="big", bufs=1))
        QT = [big.tile([128, N], f32r, name=f"qt{t}") for t in range(2)]
        KT = [big.tile([128, M], f32r, name=f"kt{t}") for t in range(2)]
        V_sb = big.tile([128, NT, GH, HEAD_DIM + 1], f32r, name="v_sb")
        # ones column for the row-sum trick: fill everything with 1.0 once;
        # the V evacuation below overwrites columns 0..63 of each (n, h)
        nc.vector.memset(V_sb.bitcast(f32), 1.0)

        ot_tiles = {}
        otpool = top.enter_context(tc.tile_pool(name="otpool", bufs=8))

        # ---- projections -------------------------------------------------
        with tc.tile_pool(name="ystream", bufs=1) as ypool:
            yt = ypool.tile([128, KC, M], f32r, name="yt")
            for c in range(KC):
                nc.scalar.dma_start(yt[:, c, :], yT[ts(c, 128), :])

            # Q^T: stream xT, all 8 psum banks accumulate over k-chunks
            with tc.tile_pool(name="xstream", bufs=3) as xpool, \
                 tc.tile_pool(name="qpsum", bufs=1, space="PSUM") as qpsum:
                qps = [qpsum.tile([128, 512], f32, name=f"qps{t}")
                       for t in range(8)]
                for c in range(KC):
                    xt = xpool.tile([128, N], f32r, name="xt")
                    nc.sync.dma_start(xt, xT[ts(c, 128), :])
                    if c == 0:
                        nc.sync.dma_start(wq_sb[:, 1:KC, :], wq_r[:, 1:KC, :])
                    for jt in range(2):
                        for ic in range(4):
                            nc.tensor.matmul(
                                qps[jt * 4 + ic],
                                wq_sb[:, c, ts(jt, 128)],
                                xt[:, ts(ic, 512)],
                                start=(c == 0), stop=(c == KC - 1))
                for jt in range(2):
                    for ic in range(4):
                        nc.vector.tensor_copy(QT[jt][:, ts(ic, 512)],
                                              qps[jt * 4 + ic])

            # K^T from resident yT
            with tc.tile_pool(name="kpsum", bufs=1, space="PSUM") as kpsum:
                kps = [kpsum.tile([128, 512], f32, name=f"kps{t}")
                       for t in range(8)]
                for c in range(KC):
                    for jt in range(2):
                        for ic in range(4):
                            nc.tensor.matmul(
                                kps[jt * 4 + ic],
                                wk_sb[:, c, ts(jt, 128)],
                                yt[:, c, ts(ic, 512)],
                                start=(c == 0), stop=(c == KC - 1))
                for jt in range(2):
                    for ic in range(4):
                        nc.vector.tensor_copy(KT[jt][:, ts(ic, 512)],
                                              kps[jt * 4 + ic])

            # V natural layout [n, j], k-chunk inner
            with tc.tile_pool(name="vpsum", bufs=4, space="PSUM") as vpsum:
                for n in range(NT):
                    vp = vpsum.tile([128, J], f32, name="vp")
                    for c in range(KC):
                        nc.tensor.matmul(
                            vp,
                            yt[:, c, ts(n, 128)],
                            wv_sb[:, c, :],
                            start=(c == 0), stop=(c == KC - 1))
                    nc.vector.tensor_copy(
                        V_sb[:, n, :, 0:HEAD_DIM],
                        vp.rearrange("p (h d) -> p h d", h=GH))

        # ---- attention ---------------------------------------------------
        # SBUF pools stay alive to the end of the kernel so the output
        # projection pools don't inherit released-zone drain dependencies
        ppool = top.enter_context(tc.tile_pool(name="ppool", bufs=4))
        rpool = top.enter_context(tc.tile_pool(name="rpool", bufs=4))
        rbpool = top.enter_context(tc.tile_pool(name="rbpool", bufs=3))
        rdram = top.enter_context(tc.tile_pool(name="rdram", bufs=3,
                                               space="DRAM"))
        with tc.tile_pool(name="spsum", bufs=2, space="PSUM") as spsum, \
             tc.tile_pool(name="opsum", bufs=1, space="PSUM") as opsum:
            for ib in range(IB):
                i_sl = ts(ib, IBS)
                oacc = {}
                for pr in range(2):
                    for lh in range(2):
                        oacc[(pr, lh)] = opsum.tile(
                            [HEAD_DIM + 1, IBS], f32, name=f"o{pr}{lh}")
                pts = {}
                for n in range(NT):
                    # S^T + exp for both head pairs of this n; the two
                    # pairs form independent ACT chains that hide each
                    # other's semaphore latency
                    for pr in range(2):
                        sp = spsum.tile([128, 2 * IBS], f32, name="sp")
                        tp_lo = dict(tile_position=(0, 0)) if PACK_S else {}
                        tp_hi = dict(tile_position=(64, 0)) if PACK_S else {}
                        nc.tensor.matmul(
                            sp[:, 0:IBS],
                            KT[pr][0:64, ts(n, 128)],
                            QT[pr][0:64, i_sl],
                            start=True, stop=True, **tp_lo)
                        nc.tensor.matmul(
                            sp[:, IBS:2 * IBS],
                            KT[pr][64:128, ts(n, 128)],
                            QT[pr][64:128, i_sl],
                            start=True, stop=True, **tp_hi)
                        pt = ppool.tile([128, 2 * IBS], f32r, name="pt")
                        nc.scalar.activation(pt, sp, Exp, bias=0.0,
                                             scale=float(SCALE))
                        pts[(pr, n)] = pt
                    # O^T accumulation for the previous n (software
                    # pipeline: keeps PE from blocking on the fresh exp)
                    if n > 0:
                        for pr in range(2):
                            pt = pts.pop((pr, n - 1))
                            for lh in range(2):
                                nc.tensor.matmul(
                                    oacc[(pr, lh)],
                                    V_sb[:, n - 1, 2 * pr + lh, :],
                                    pt[:, lh * IBS:(lh + 1) * IBS],
                                    start=(n - 1 == 0), stop=False)
                for pr in range(2):
                    pt = pts.pop((pr, NT - 1))
                    for lh in range(2):
                        nc.tensor.matmul(
                            oacc[(pr, lh)],
                            V_sb[:, NT - 1, 2 * pr + lh, :],
                            pt[:, lh * IBS:(lh + 1) * IBS],
                            start=False, stop=True)
                # PSUM-releasing evacuations for both pairs first (the
                # next round's O matmuls wait on these slots), then the
                # off-critical-path normalize chains.
                blk = {}
                for pr in range(2):
                    o_lo, o_hi = oacc[(pr, 0)], oacc[(pr, 1)]
                    ot = otpool.tile([128, IBS], f32r, name="ot")
                    nc.vector.tensor_copy(ot[0:64, :], o_lo[0:64, :])
                    nc.vector.tensor_copy(ot[64:128, :], o_hi[0:64, :])
                    rs_lo = rpool.tile([1, IBS], f32, name="rslo")
                    rs_hi = rpool.tile([1, IBS], f32, name="rshi")
                    nc.vector.tensor_copy(rs_lo, o_lo[64:65, :])
                    nc.vector.tensor_copy(rs_hi, o_hi[64:65, :])
                    blk[pr] = (ot, rs_lo, rs_hi)
                for pr in range(2):
                    ot, rs_lo, rs_hi = blk[pr]
                    rd = rdram.tile([2, IBS], f32, name="rd")
                    nc.sync.dma_start(rd[0:1, :], rs_lo)
                    nc.sync.dma_start(rd[1:2, :], rs_hi)
                    rb = rbpool.tile([128, IBS], f32, name="rb")
                    nc.sync.dma_start(rb[0:64, :],
                                      rd[0:1, :].partition_broadcast(64))
                    nc.sync.dma_start(rb[64:128, :],
                                      rd[1:2, :].partition_broadcast(64))
                    rb2 = rbpool.tile([128, IBS], f32, name="rb2")
                    nc.vector.reciprocal_approx_fast(rb2, rb)
                    nc.vector.tensor_mul(ot, ot.bitcast(f32), rb2)
                    ot_tiles[(ib, pr)] = ot

        # ---- output projection ------------------------------------------
        obpool = top.enter_context(tc.tile_pool(name="obpool", bufs=4))
        with tc.tile_pool(name="oppsum", bufs=4, space="PSUM") as oppsum:
            for ib in range(IB):
                for icr in range(IBS // 128):
                    for cc in range(DIM // 512):
                        op = oppsum.tile([128, 512], f32, name="op")
                        for jt in range(2):
                            nc.tensor.matmul(
                                op,
                                ot_tiles[(ib, jt)][:, ts(icr, 128)],
                                wp_sb[:, jt, ts(cc, 512)],
                                start=(jt == 0), stop=(jt == 1))
                        ob = obpool.tile([128, 512], f32, name="ob")
                        nc.scalar.copy(ob, op)
                        nc.sync.dma_start(
                            out[ds(ib * IBS + icr * 128, 128), ts(cc, 512)],
                            ob)

    nc.compile()
    return nc


def _get_nc():
    global _NC
    if _NC is None:
        _NC = _build()
    return _NC


def _shard_inputs(x, y, Wq, Wk, Wv, Wp):
    x = np.asarray(x, np.float32)
    y = np.asarray(y, np.float32)
    Wq = np.asarray(Wq, np.float32)
    Wk = np.asarray(Wk, np.float32)
    Wv = np.asarray(Wv, np.float32)
    Wp = np.asarray(Wp, np.float32)
    xT = [np.ascontiguousarray(x[b].T) for b in range(B)]
    yT = [np.ascontiguousarray(y[b].T) for b in range(B)]
    in_maps = []
    for c in range(NCORES):
        b, g = divmod(c, NCORES // B)
        sl = slice(g * J, (g + 1) * J)
        in_maps.append({
            "xT": xT[b],
            "yT": yT[b],
            "wq": np.ascontiguousarray(Wq[:, sl]),
            "wk": np.ascontiguousarray(Wk[:, sl]),
            "wv": np.ascontiguousarray(Wv[:, sl]),
            "wp": np.ascontiguousarray(Wp[sl, :]),
        })
    return in_maps


def run(inputs, trace=False, **spmd_kwargs):
    from concourse.bass_utils import run_bass_kernel_spmd
    nc = _get_nc()
    in_maps = _shard_inputs(inputs["x"], inputs["y"], inputs["Wq"],
                            inputs["Wk"], inputs["Wv"], inputs["Wp"])
    res = run_bass_kernel_spmd(nc, in_maps, core_ids=list(range(NCORES)),
                               trace=trace, **spmd_kwargs)
    bp = np.asarray(inputs["bp"], np.float32)
    gpb = NCORES // B
    full = np.empty((B, N, DIM), np.float32)
    for b in range(B):
        acc = res.results[b * gpb]["out"].astype(np.float32)
        for g in range(1, gpb):
            acc = acc + res.results[b * gpb + g]["out"]
        full[b] = acc + bp
    return full, res


def kernel(**inputs):
    out, _ = run(inputs, trace=False)
    return out



# revision 3
# speedup vs baseline: 1.0160x; 1.0160x over previous
"""Cross-attention kernel for 8 Trainium2 NeuronCores.

Sharding: data-parallel over batch (B=2) x tensor-parallel over heads
(16 heads -> 4 groups of 4 heads).  Core c handles batch c//4, head
group c%4.  Each core computes, for its 4 heads:
    Q^T = Wq_g^T x_b^T        [256, 2048]   (d-on-partitions layout)
    K^T = Wk_g^T y_b^T        [256, 2048]
    V   = y_b Wv_g            [2048, 256]   (n-on-partitions layout)
    S^T_h = K_h Q_h^T / 8; P^T = exp(S^T)
    O^T_h (+row sums via a ones-column in V) = [V_h|1]^T P^T
    partial = (O^T/rowsum)^T Wp_g           [2048, 1024]
The 4 partials per batch are summed on the host and bp is added.

All matmul operands are bf16 (host-cast): same PE rate as fp32r but
half the HBM traffic and SBUF footprint; accumulation stays fp32 in
PSUM.  The attention loop runs one head-pair (pr) per pass so PSUM
fits in 6 banks, leaving 2 banks to interleave the V projection into
i-block 0 and the output projection into later i-blocks -- no serial
projection phases.  The softmax exp on ScalarE (1024 elems/instr,
~1.1us) is the steady-state bottleneck; emission keeps one exp in
flight per S-group so ScalarE never starves.
"""

import numpy as np

B = 2
N = 2048          # query sequence length
M = 2048          # key sequence length
DIM = 1024
HEAD_DIM = 64
SCALE = HEAD_DIM ** -0.5
NCORES = 8
GH = 4            # heads per core
J = GH * HEAD_DIM # 256 projected columns per core
KC = DIM // 128   # 8 contraction chunks
NT = M // 128     # 16 key tiles
IBS = 512         # i-block size
IB = N // IBS     # 4 i-blocks

_NC = None


def _build():
    from contextlib import ExitStack

    import concourse.bass as bass
    import concourse.tile as tile
    from concourse import bacc, mybir
    from concourse.bass import ts, ds

    f32 = mybir.dt.float32
    bf16 = mybir.dt.bfloat16
    Exp = mybir.ActivationFunctionType.Exp

    nc = bacc.Bacc("TRN2", target_bir_lowering=False, debug=False,
                   num_devices=NCORES)
    xT = nc.dram_tensor("xT", [DIM, N], bf16, kind="ExternalInput").ap()
    yT = nc.dram_tensor("yT", [DIM, M], bf16, kind="ExternalInput").ap()
    wq = nc.dram_tensor("wq", [DIM, J], bf16, kind="ExternalInput").ap()
    wk = nc.dram_tensor("wk", [DIM, J], bf16, kind="ExternalInput").ap()
    wv = nc.dram_tensor("wv", [DIM, J], bf16, kind="ExternalInput").ap()
    wp = nc.dram_tensor("wp", [J, DIM], bf16, kind="ExternalInput").ap()
    out = nc.dram_tensor("out", [N, DIM], f32, kind="ExternalOutput").ap()

    with tile.TileContext(nc) as tc, ExitStack() as top:
        wpool = top.enter_context(tc.tile_pool(name="weights", bufs=1))
        wq_sb = wpool.tile([128, KC, J], bf16, name="wq_sb")
        wk_sb = wpool.tile([128, KC, J], bf16, name="wk_sb")
        wv_sb = wpool.tile([128, KC, J], bf16, name="wv_sb")
        wp_sb = wpool.tile([128, 2, DIM], bf16, name="wp_sb")
        # queue split: x + wq stream on the SP HWDGE ring, y on the Act
        # HWDGE ring, remaining weights + output writes on gpsimd SWDGE
        wq_r = wq.rearrange("(c p) j -> p c j", p=128)
        nc.sync.dma_start(wq_sb[:, 0, :], wq_r[:, 0, :])
        nc.gpsimd.dma_start(wk_sb, wk.rearrange("(c p) j -> p c j", p=128))
        nc.gpsimd.dma_start(wv_sb, wv.rearrange("(c p) j -> p c j", p=128))
        nc.gpsimd.dma_start(wp_sb, wp.rearrange("(t p) c -> p t c", p=128))

        big = top.enter_context(tc.tile_pool(name="big", bufs=1))
        QT = [big.tile([128, N], bf16, name=f"qt{t}") for t in range(2)]
        KT = [big.tile([128, M], bf16, name=f"kt{t}") for t in range(2)]
        V_sb = big.tile([128, NT, GH, HEAD_DIM + 1], bf16, name="v_sb")
        # ones column for the row-sum trick; V evac fills cols 0..63
        nc.vector.memset(V_sb[:, :, :, HEAD_DIM:HEAD_DIM + 1], 1.0)

        ypool = top.enter_context(tc.tile_pool(name="ystream", bufs=1))
        yt = ypool.tile([128, KC, M], bf16, name="yt")
        for c in range(KC):
            nc.scalar.dma_start(yt[:, c, :], yT[ts(c, 128), :])

        # ---- Q/K projections --------------------------------------------
        with tc.tile_pool(name="xstream", bufs=3) as xpool, \
             tc.tile_pool(name="qpsum", bufs=1, space="PSUM") as qpsum:
            qps = [qpsum.tile([128, 512], f32, name=f"qps{t}")
                   for t in range(8)]
            for c in range(KC):
                xt = xpool.tile([128, N], bf16, name="xt")
                nc.sync.dma_start(xt, xT[ts(c, 128), :])
                if c == 0:
                    nc.sync.dma_start(wq_sb[:, 1:KC, :], wq_r[:, 1:KC, :])
                for jt in range(2):
                    for ic in range(4):
                        nc.tensor.matmul(
                            qps[jt * 4 + ic],
                            wq_sb[:, c, ts(jt, 128)],
                            xt[:, ts(ic, 512)],
                            start=(c == 0), stop=(c == KC - 1))
            # ic-major evac so the first i-block's columns land first
            for ic in range(4):
                for jt in range(2):
                    nc.vector.tensor_copy(QT[jt][:, ts(ic, 512)],
                                          qps[jt * 4 + ic])

        with tc.tile_pool(name="kpsum", bufs=1, space="PSUM") as kpsum:
            kps = [kpsum.tile([128, 512], f32, name=f"kps{t}")
                   for t in range(8)]
            for c in range(KC):
                for jt in range(2):
                    for ic in range(4):
                        nc.tensor.matmul(
                            kps[jt * 4 + ic],
                            wk_sb[:, c, ts(jt, 128)],
                            yt[:, c, ts(ic, 512)],
                            start=(c == 0), stop=(c == KC - 1))
            for ic in range(4):
                for jt in range(2):
                    nc.vector.tensor_copy(KT[jt][:, ts(ic, 512)],
                                          kps[jt * 4 + ic])

        # ---- attention with interleaved V / output projection -----------
        ppool = top.enter_context(tc.tile_pool(name="ppool", bufs=3))
        otpool = top.enter_context(tc.tile_pool(name="otpool", bufs=2))
        otbpool = top.enter_context(tc.tile_pool(name="otbpool", bufs=4))
        rpool = top.enter_context(tc.tile_pool(name="rpool", bufs=4))
        rbpool = top.enter_context(tc.tile_pool(name="rbpool", bufs=3))
        rdram = top.enter_context(tc.tile_pool(name="rdram", bufs=3,
                                               space="DRAM"))
        obpool = top.enter_context(tc.tile_pool(name="obpool", bufs=4))
        spsum = top.enter_context(
            tc.tile_pool(name="spsum", bufs=2, space="PSUM"))
        opsum = top.enter_context(
            tc.tile_pool(name="opsum", bufs=1, space="PSUM"))
        vpsum = top.enter_context(
            tc.tile_pool(name="vpsum", bufs=1, space="PSUM"))
        oppsum = top.enter_context(
            tc.tile_pool(name="oppsum", bufs=1, space="PSUM"))

        ot_tiles = {}

        def v_tile(n):
            vp = vpsum.tile([128, J], f32, name="vp")
            for c in range(KC):
                nc.tensor.matmul(
                    vp,
                    yt[:, c, ts(n, 128)],
                    wv_sb[:, c, :],
                    start=(c == 0), stop=(c == KC - 1))
            nc.vector.tensor_copy(
                V_sb[:, n, :, 0:HEAD_DIM],
                vp.rearrange("p (h d) -> p h d", h=GH))

        def proj_unit(ib, u):
            icr, cc = divmod(u, 2)
            op = oppsum.tile([128, 512], f32, name="op")
            for jt in range(2):
                nc.tensor.matmul(
                    op,
                    ot_tiles[(ib, jt)][:, ts(icr, 128)],
                    wp_sb[:, jt, ts(cc, 512)],
                    start=(jt == 0), stop=(jt == 1))
            ob = obpool.tile([128, 512], f32, name="ob")
            nc.vector.tensor_copy(ob, op)
            nc.gpsimd.dma_start(
                out[ds(ib * IBS + icr * 128, 128), ts(cc, 512)], ob)

        # V for n=0,1 must exist before the first O matmuls of i-block 0
        v_tile(0)
        v_tile(1)

        for ib in range(IB):
            i_sl = ts(ib, IBS)
            for pr in range(2):
                oacc = [opsum.tile([HEAD_DIM + 1, IBS], f32, name=f"o{lh}")
                        for lh in range(2)]
                pts = {}
                for n in range(NT):
                    sp = spsum.tile([128, 2 * IBS], f32, name="sp")
                    nc.tensor.matmul(
                        sp[:, 0:IBS],
                        KT[pr][0:64, ts(n, 128)],
                        QT[pr][0:64, i_sl],
                        start=True, stop=True, tile_position=(0, 0))
                    nc.tensor.matmul(
                        sp[:, IBS:2 * IBS],
                        KT[pr][64:128, ts(n, 128)],
                        QT[pr][64:128, i_sl],
                        start=True, stop=True, tile_position=(64, 0))
                    pt = ppool.tile([128, 2 * IBS], bf16, name="pt")
                    nc.scalar.activation(pt, sp, Exp, bias=0.0,
                                         scale=float(SCALE))
                    pts[n] = pt
                    if n > 0:
                        pt_prev = pts.pop(n - 1)
                        for lh in range(2):
                            nc.tensor.matmul(
                                oacc[lh],
                                V_sb[:, n - 1, 2 * pr + lh, :],
                                pt_prev[:, lh * IBS:(lh + 1) * IBS],
                                start=(n - 1 == 0), stop=False)
                    # PE fillers: V tiles during (ib0, pr0); the previous
                    # i-block's output projection during pass 0 of ib>=1
                    if ib == 0 and pr == 0 and n < 14:
                        v_tile(n + 2)
                    if ib >= 1 and pr == 0 and 3 <= n <= 10:
                        proj_unit(ib - 1, n - 3)
                pt_prev = pts.pop(NT - 1)
                for lh in range(2):
                    nc.tensor.matmul(
                        oacc[lh],
                        V_sb[:, NT - 1, 2 * pr + lh, :],
                        pt_prev[:, lh * IBS:(lh + 1) * IBS],
                        start=False, stop=True)
                # PSUM-releasing evacuations, then the normalize chain
                ot = otpool.tile([128, IBS], f32, name="ot")
                nc.vector.tensor_copy(ot[0:64, :], oacc[0][0:64, :])
                nc.vector.tensor_copy(ot[64:128, :], oacc[1][0:64, :])
                rs_lo = rpool.tile([1, IBS], f32, name="rslo")
                rs_hi = rpool.tile([1, IBS], f32, name="rshi")
                nc.vector.tensor_copy(rs_lo, oacc[0][64:65, :])
                nc.vector.tensor_copy(rs_hi, oacc[1][64:65, :])
                rd = rdram.tile([2, IBS], f32, name="rd")
                nc.sync.dma_start(rd[0:1, :], rs_lo)
                nc.sync.dma_start(rd[1:2, :], rs_hi)
                rb = rbpool.tile([128, IBS], f32, name="rb")
                nc.sync.dma_start(rb[0:64, :],
                                  rd[0:1, :].partition_broadcast(64))
                nc.sync.dma_start(rb[64:128, :],
                                  rd[1:2, :].partition_broadcast(64))
                rb2 = rbpool.tile([128, IBS], f32, name="rb2")
                nc.vector.reciprocal_approx_fast(rb2, rb)
                otb = otbpool.tile([128, IBS], bf16, name="otb")
                nc.vector.tensor_mul(otb, ot, rb2)
                ot_tiles[(ib, pr)] = otb

        # last i-block's projection has no later pass to hide in
        for u in range(8):
            proj_unit(IB - 1, u)

    nc.compile()
    return nc


def _get_nc():
    global _NC
    if _NC is None:
        _NC = _build()
    return _NC


def _shard_inputs(x, y, Wq, Wk, Wv, Wp):
    import ml_dtypes
    bf16 = ml_dtypes.bfloat16
    x = np.asarray(x, np.float32)
    y = np.asarray(y, np.float32)
    xT = [np.ascontiguousarray(x[b].T.astype(bf16)) for b in range(B)]
    yT = [np.ascontiguousarray(y[b].T.astype(bf16)) for b in range(B)]
    Wq = np.asarray(Wq, np.float32).astype(bf16)
    Wk = np.asarray(Wk, np.float32).astype(bf16)
    Wv = np.asarray(Wv, np.float32).astype(bf16)
    Wp = np.asarray(Wp, np.float32).astype(bf16)
    in_maps = []
    for c in range(NCORES):
        b, g = divmod(c, NCORES // B)
        sl = slice(g * J, (g + 1) * J)
        in_maps.append({
            "xT": xT[b],
            "yT": yT[b],
            "wq": np.ascontiguousarray(Wq[:, sl]),
            "wk": np.ascontiguousarray(Wk[:, sl]),
            "wv": np.ascontiguousarray(Wv[:, sl]),
            "wp": np.ascontiguousarray(Wp[sl, :]),
        })
    return in_maps


def run(inputs, trace=False, **spmd_kwargs):
    from concourse.bass_utils import run_bass_kernel_spmd
    nc = _get_nc()
    in_maps = _shard_inputs(inputs["x"], inputs["y"], inputs["Wq"],
                            inputs["Wk"], inputs["Wv"], inputs["Wp"])
    res = run_bass_kernel_spmd(nc, in_maps, core_ids=list(range(NCORES)),
                               trace=trace, **spmd_kwargs)
    bp = np.asarray(inputs["bp"], np.float32)
    gpb = NCORES // B
    full = np.empty((B, N, DIM), np.float32)
    for b in range(B):
        acc = res.results[b * gpb]["out"].astype(np.float32)
        for g in range(1, gpb):
            acc = acc + res.results[b * gpb + g]["out"]
        full[b] = acc + bp
    return full, res


def kernel(**inputs):
    out, _ = run(inputs, trace=False)
    return out


# revision 15
# speedup vs baseline: 1.0757x; 1.0587x over previous
"""Cross-attention kernel for 8 Trainium2 NeuronCores.

Sharding: data-parallel over batch (B=2) x tensor-parallel over heads
(16 heads -> 4 groups of 4 heads).  Core c handles batch c//4, head
group c%4.  Each core computes, for its 4 heads:
    Q^T = Wq_g^T x_b^T        [256, 2048]   (d-on-partitions layout)
    K^T = Wk_g^T y_b^T        [256, 2048]
    V   = y_b Wv_g            [2048, 256]   (n-on-partitions layout)
    S^T_h = K_h Q_h^T / 8; P^T = exp(S^T)
    O^T_h (+row sums via a ones-column in V) = [V_h|1]^T P^T
    partial = (O^T/rowsum)^T Wp_g           [2048, 1024]
The 4 partials per batch are summed on the host and bp is added.

All matmul operands are bf16 (host-cast): same PE rate as fp32r but
half the HBM traffic and SBUF footprint; accumulation stays fp32 in
PSUM.  The attention loop runs one head-pair (pr) per pass so PSUM
fits in 6 banks, leaving 2 banks to interleave the V projection into
i-block 0 and the output projection into later i-blocks -- no serial
projection phases.  The softmax exp on ScalarE (1024 elems/instr,
~1.1us) is the steady-state bottleneck; emission keeps one exp in
flight per S-group so ScalarE never starves.
"""

import numpy as np

B = 2
N = 2048          # query sequence length
M = 2048          # key sequence length
DIM = 1024
HEAD_DIM = 64
SCALE = HEAD_DIM ** -0.5
NCORES = 8
GH = 4            # heads per core
J = GH * HEAD_DIM # 256 projected columns per core
KC = DIM // 128   # 8 contraction chunks
NT = M // 128     # 16 key tiles
IBS = 512         # i-block size
IB = N // IBS     # 4 i-blocks

_NC = None


def _build():
    from contextlib import ExitStack

    import concourse.bass as bass
    import concourse.tile as tile
    from concourse import bacc, mybir
    from concourse.bass import ts, ds

    f32 = mybir.dt.float32
    bf16 = mybir.dt.bfloat16
    Exp = mybir.ActivationFunctionType.Exp

    nc = bacc.Bacc("TRN2", target_bir_lowering=False, debug=False,
                   num_devices=NCORES)
    xT = nc.dram_tensor("xT", [DIM, N], bf16, kind="ExternalInput").ap()
    yT = nc.dram_tensor("yT", [DIM, M], bf16, kind="ExternalInput").ap()
    wq = nc.dram_tensor("wq", [DIM, J], bf16, kind="ExternalInput").ap()
    wk = nc.dram_tensor("wk", [DIM, J], bf16, kind="ExternalInput").ap()
    wv = nc.dram_tensor("wv", [DIM, J], bf16, kind="ExternalInput").ap()
    wp = nc.dram_tensor("wp", [J, DIM], bf16, kind="ExternalInput").ap()
    out = nc.dram_tensor("out", [N, DIM], f32, kind="ExternalOutput").ap()

    with tile.TileContext(nc) as tc, ExitStack() as top:
        wpool = top.enter_context(tc.tile_pool(name="weights", bufs=1))
        wq_sb = wpool.tile([128, KC, J], bf16, name="wq_sb")
        wk_sb = wpool.tile([128, KC, J], bf16, name="wk_sb")
        wv_sb = wpool.tile([128, KC, J], bf16, name="wv_sb")
        wp_sb = wpool.tile([128, 2, DIM], bf16, name="wp_sb")
        # queue split: wq + x then wk/wv/wp on the SP HWDGE ring, y alone
        # on the Act HWDGE ring -- the two rings split HBM bandwidth and
        # the weights trail x so they don't steal from the critical path.
        # Output writes go on the gpsimd SWDGE ring.
        wq_r = wq.rearrange("(c p) j -> p c j", p=128)
        nc.sync.dma_start(wq_sb[:, 0, :], wq_r[:, 0, :])

        big = top.enter_context(tc.tile_pool(name="big", bufs=1))
        QT = [big.tile([128, N], bf16, name=f"qt{t}") for t in range(2)]
        KT = [big.tile([128, M], bf16, name=f"kt{t}") for t in range(2)]
        V_sb = big.tile([128, NT, GH, HEAD_DIM + 1], bf16, name="v_sb")
        # ones column for the row-sum trick; V evac fills cols 0..63
        nc.vector.memset(V_sb[:, :, :, HEAD_DIM:HEAD_DIM + 1], 1.0)

        ypool = top.enter_context(tc.tile_pool(name="ystream", bufs=1))
        yt = ypool.tile([128, KC, M], bf16, name="yt")
        for c in range(KC):
            nc.scalar.dma_start(yt[:, c, :], yT[ts(c, 128), :])

        # ---- Q/K projections --------------------------------------------
        xt = ypool.tile([128, KC, N], bf16, name="xt")
        for c in range(KC):
            nc.sync.dma_start(xt[:, c, :], xT[ts(c, 128), :])
            if c == 0:
                nc.sync.dma_start(wq_sb[:, 1:KC, :], wq_r[:, 1:KC, :])
        # weights trail the x stream on the sync ring
        nc.sync.dma_start(wk_sb, wk.rearrange("(c p) j -> p c j", p=128))
        nc.sync.dma_start(wv_sb, wv.rearrange("(c p) j -> p c j", p=128))
        nc.sync.dma_start(wp_sb, wp.rearrange("(t p) c -> p t c", p=128))

        with tc.tile_pool(name="qpsum", bufs=1, space="PSUM") as qpsum:
            qps = [qpsum.tile([128, 512], f32, name=f"qps{t}")
                   for t in range(8)]
            for c in range(KC):
                for jt in range(2):
                    for ic in range(4):
                        nc.tensor.matmul(
                            qps[jt * 4 + ic],
                            wq_sb[:, c, ts(jt, 128)],
                            xt[:, c, ts(ic, 512)],
                            start=(c == 0), stop=(c == KC - 1))
            # ic-major evac so the first i-block's columns land first
            for ic in range(4):
                for jt in range(2):
                    nc.vector.tensor_copy(QT[jt][:, ts(ic, 512)],
                                          qps[jt * 4 + ic])

        with tc.tile_pool(name="kpsum", bufs=1, space="PSUM") as kpsum:
            kps = [kpsum.tile([128, 512], f32, name=f"kps{t}")
                   for t in range(8)]
            for c in range(KC):
                for jt in range(2):
                    for ic in range(4):
                        nc.tensor.matmul(
                            kps[jt * 4 + ic],
                            wk_sb[:, c, ts(jt, 128)],
                            yt[:, c, ts(ic, 512)],
                            start=(c == 0), stop=(c == KC - 1))
            for ic in range(4):
                for jt in range(2):
                    nc.vector.tensor_copy(KT[jt][:, ts(ic, 512)],
                                          kps[jt * 4 + ic])

        # ---- attention with interleaved V / output projection -----------
        ppool = top.enter_context(tc.tile_pool(name="ppool", bufs=3))
        otpool = top.enter_context(tc.tile_pool(name="otpool", bufs=2))
        otbpool = top.enter_context(tc.tile_pool(name="otbpool", bufs=4))
        rpool = top.enter_context(tc.tile_pool(name="rpool", bufs=4))
        rbpool = top.enter_context(tc.tile_pool(name="rbpool", bufs=3))
        rdram = top.enter_context(tc.tile_pool(name="rdram", bufs=3,
                                               space="DRAM"))
        obpool = top.enter_context(tc.tile_pool(name="obpool", bufs=4))
        spsum = top.enter_context(
            tc.tile_pool(name="spsum", bufs=2, space="PSUM"))
        opsum = top.enter_context(
            tc.tile_pool(name="opsum", bufs=1, space="PSUM"))
        # V (i-block 0) and the output projection (i-blocks 1+) never
        # overlap, so they share one double-buffered PSUM pool
        aux = top.enter_context(
            tc.tile_pool(name="aux", bufs=2, space="PSUM"))

        ot_tiles = {}

        def v_tile(n):
            vp = aux.tile([128, 512], f32, name="ax")[:, 0:J]
            for c in range(KC):
                nc.tensor.matmul(
                    vp,
                    yt[:, c, ts(n, 128)],
                    wv_sb[:, c, :],
                    start=(c == 0), stop=(c == KC - 1))
            nc.vector.tensor_copy(
                V_sb[:, n, :, 0:HEAD_DIM],
                vp.rearrange("p (h d) -> p h d", h=GH))

        def proj_unit(ib, u):
            icr, cc = divmod(u, 2)
            op = aux.tile([128, 512], f32, name="ax")
            for jt in range(2):
                nc.tensor.matmul(
                    op,
                    ot_tiles[(ib, jt)][:, ts(icr, 128)],
                    wp_sb[:, jt, ts(cc, 512)],
                    start=(jt == 0), stop=(jt == 1))
            ob = obpool.tile([128, 512], f32, name="ob")
            nc.vector.tensor_copy(ob, op)
            nc.gpsimd.dma_start(
                out[ds(ib * IBS + icr * 128, 128), ts(cc, 512)], ob)

        # V for n=0,1 must exist before the first O matmuls of i-block 0
        v_tile(0)
        v_tile(1)

        for ib in range(IB):
            i_sl = ts(ib, IBS)
            for pr in range(2):
                oacc = [opsum.tile([HEAD_DIM + 1, IBS], f32, name=f"o{lh}")
                        for lh in range(2)]
                pts = {}
                for n in range(NT):
                    sp = spsum.tile([128, 2 * IBS], f32, name="sp")
                    nc.tensor.matmul(
                        sp[:, 0:IBS],
                        KT[pr][0:64, ts(n, 128)],
                        QT[pr][0:64, i_sl],
                        start=True, stop=True, tile_position=(0, 0))
                    nc.tensor.matmul(
                        sp[:, IBS:2 * IBS],
                        KT[pr][64:128, ts(n, 128)],
                        QT[pr][64:128, i_sl],
                        start=True, stop=True, tile_position=(64, 0))
                    pt = ppool.tile([128, 2 * IBS], bf16, name="pt")
                    nc.scalar.activation(pt, sp, Exp, bias=0.0,
                                         scale=float(SCALE))
                    pts[n] = pt
                    if n > 0:
                        pt_prev = pts.pop(n - 1)
                        for lh in range(2):
                            nc.tensor.matmul(
                                oacc[lh],
                                V_sb[:, n - 1, 2 * pr + lh, :],
                                pt_prev[:, lh * IBS:(lh + 1) * IBS],
                                start=(n - 1 == 0), stop=False)
                    # PE fillers: V tiles during (ib0, pr0); the previous
                    # i-block's output projection during pass 0 of ib>=1
                    if ib == 0 and pr == 0 and n < 14:
                        v_tile(n + 2)
                    if ib >= 1 and pr == 0 and 3 <= n <= 10:
                        proj_unit(ib - 1, n - 3)
                pt_prev = pts.pop(NT - 1)
                for lh in range(2):
                    nc.tensor.matmul(
                        oacc[lh],
                        V_sb[:, NT - 1, 2 * pr + lh, :],
                        pt_prev[:, lh * IBS:(lh + 1) * IBS],
                        start=False, stop=True)
                # row sums first so the broadcast roundtrip starts early;
                # the ot evacs then release the PSUM banks
                rs_lo = rpool.tile([1, IBS], f32, name="rslo")
                rs_hi = rpool.tile([1, IBS], f32, name="rshi")
                nc.vector.tensor_copy(rs_lo, oacc[0][64:65, :])
                nc.vector.tensor_copy(rs_hi, oacc[1][64:65, :])
                rd = rdram.tile([2, IBS], f32, name="rd")
                nc.sync.dma_start(rd[0:1, :], rs_lo)
                nc.sync.dma_start(rd[1:2, :], rs_hi)
                ot = otpool.tile([128, IBS], f32, name="ot")
                nc.vector.tensor_copy(ot[0:64, :], oacc[0][0:64, :])
                nc.vector.tensor_copy(ot[64:128, :], oacc[1][0:64, :])
                rb = rbpool.tile([128, IBS], f32, name="rb")
                nc.sync.dma_start(rb[0:64, :],
                                  rd[0:1, :].partition_broadcast(64))
                nc.sync.dma_start(rb[64:128, :],
                                  rd[1:2, :].partition_broadcast(64))
                rb2 = rbpool.tile([128, IBS], f32, name="rb2")
                nc.vector.reciprocal_approx_fast(rb2, rb)
                otb = otbpool.tile([128, IBS], bf16, name="otb")
                nc.vector.tensor_mul(otb, ot, rb2)
                ot_tiles[(ib, pr)] = otb

        # last i-block's projection has no later pass to hide in
        for u in range(8):
            proj_unit(IB - 1, u)

    nc.compile()
    return nc


def _get_nc():
    global _NC
    if _NC is None:
        _NC = _build()
    return _NC


def _shard_inputs(x, y, Wq, Wk, Wv, Wp):
    import ml_dtypes
    bf16 = ml_dtypes.bfloat16
    x = np.asarray(x, np.float32)
    y = np.asarray(y, np.float32)
    xT = [np.ascontiguousarray(x[b].T.astype(bf16)) for b in range(B)]
    yT = [np.ascontiguousarray(y[b].T.astype(bf16)) for b in range(B)]
    Wq = np.asarray(Wq, np.float32).astype(bf16)
    Wk = np.asarray(Wk, np.float32).astype(bf16)
    Wv = np.asarray(Wv, np.float32).astype(bf16)
    Wp = np.asarray(Wp, np.float32).astype(bf16)
    in_maps = []
    for c in range(NCORES):
        b, g = divmod(c, NCORES // B)
        sl = slice(g * J, (g + 1) * J)
        in_maps.append({
            "xT": xT[b],
            "yT": yT[b],
            "wq": np.ascontiguousarray(Wq[:, sl]),
            "wk": np.ascontiguousarray(Wk[:, sl]),
            "wv": np.ascontiguousarray(Wv[:, sl]),
            "wp": np.ascontiguousarray(Wp[sl, :]),
        })
    return in_maps


def run(inputs, trace=False, **spmd_kwargs):
    from concourse.bass_utils import run_bass_kernel_spmd
    nc = _get_nc()
    in_maps = _shard_inputs(inputs["x"], inputs["y"], inputs["Wq"],
                            inputs["Wk"], inputs["Wv"], inputs["Wp"])
    res = run_bass_kernel_spmd(nc, in_maps, core_ids=list(range(NCORES)),
                               trace=trace, **spmd_kwargs)
    bp = np.asarray(inputs["bp"], np.float32)
    gpb = NCORES // B
    full = np.empty((B, N, DIM), np.float32)
    for b in range(B):
        acc = res.results[b * gpb]["out"].astype(np.float32)
        for g in range(1, gpb):
            acc = acc + res.results[b * gpb + g]["out"]
        full[b] = acc + bp
    return full, res


def kernel(**inputs):
    out, _ = run(inputs, trace=False)
    return out
